# revision 8
# baseline (speedup 1.0000x reference)
"""Trainium2 Bass kernel for a Mixtral decoder layer (attention + top-2 MoE).

Contract: kernel(**inputs) takes the FULL unsharded inputs (as produced by
reference.setup_inputs()) and returns the full outputs (out, residual), both
[B, S, D] float32.

Sharding across the 8 NeuronCores:
  Phase 1 (attention): tensor-parallel over heads. Each core owns 2 q-heads +
  1 kv-head (colwise qkv slice) and the matching 256-column slice of wo
  (rowwise o_proj). Cores emit o_proj partial sums [T, D]; the host combines
  them (the all-reduce step) and applies the residual add + post-attention
  RMSNorm + router on the host (tiny fraction of total FLOPs).
  Phase 2 (MoE): expert-parallel. Core e owns expert e's weights; the host
  gathers the tokens routed to each expert (capacity-padded), each core runs
  the SwiGLU expert densely, and the host scatter-adds the weighted results.

Matmuls run in float32r (TF32-like, ~1.5e-4 rel err) at full PE rate.
"""

import math
from functools import lru_cache

import numpy as np

import concourse.bass as bass
import concourse.mybir as mybir
import concourse.tile as tile
from concourse import bacc
from concourse import bass_utils

# ---- problem shapes (hardcoded per contract) ----
B, S, D = 2, 2048, 2048
NH, NKV, HD = 16, 8, 128
E, TOPK, F = 8, 2, 4096
EPS = 1e-5
T = B * S
NCORES = 8
P = 128

F32 = mybir.dt.float32
F32R = mybir.dt.float32r
DKT = D // P   # 16 k-tiles over D
FBT = F // P   # 32 f-blocks over F
FG = 4         # f-blocks per group in phase 2 (psum-accumulated w2)


def _chunks(n, lo=256, hi=512):
    """Split n (multiple of 128, >=lo) into chunks in [lo, hi], multiples of 128."""
    out = []
    rem = n
    while rem > 0:
        if rem <= hi:
            out.append(rem)
            break
        if rem - hi >= lo:
            out.append(hi)
            rem -= hi
        else:
            c = rem - lo
            out.append(c)
            rem -= c
    assert all(lo <= c <= hi and c % 128 == 0 for c in out) and sum(out) == n, (n, out)
    return out


# ---------------------------------------------------------------- phase 2
@lru_cache(maxsize=None)
def build_phase2(C, reps=1, sim_safe=False):
    """Per-core SwiGLU expert over C capacity-padded tokens.

    Inputs (per core): xt [D, C] f32r, w1t/w3t [D, F] f32r (= w1[e].T),
    w2t [F, D] f32r (= w2[e].T). Output: y [D, C] f32 (= expert(x).T).
    """
    CH = C // 2
    nch = []
    off = 0
    for c in _chunks(CH):
        nch.append((off, c))
        off += c

    nc = bacc.Bacc(None, target_bir_lowering=False, debug=False)
    with tile.TileContext(nc) as tc:
        with (
            tc.tile_pool(name="dram", bufs=1, space="DRAM") as dram,
            tc.tile_pool(name="xp", bufs=1) as xp,
            tc.tile_pool(name="yp", bufs=1) as yp,
            tc.tile_pool(name="wp", bufs=2) as wp,
            tc.tile_pool(name="gup", bufs=2) as gup,
            tc.tile_pool(name="gtmp", bufs=3) as gtmp,
            tc.tile_pool(name="ps_g", bufs=2, space="PSUM") as ps_g,
            tc.tile_pool(name="ps_y", bufs=2, space="PSUM") as ps_y,
        ):
            xt = dram.tile([D, C], F32R, kind="ExternalInput", name="xt", uniquify=False)
            w1t = dram.tile([D, F], F32R, kind="ExternalInput", name="w1t", uniquify=False)
            w3t = dram.tile([D, F], F32R, kind="ExternalInput", name="w3t", uniquify=False)
            w2t = dram.tile([F, D], F32R, kind="ExternalInput", name="w2t", uniquify=False)
            y = dram.tile([D, C], F32, kind="ExternalOutput", name="y", uniquify=False)

            def body():
                for h in range(2):
                    x_sb = xp.tile([P, DKT, CH], F32R, tag="x")
                    nc.sync.dma_start(
                        x_sb[:],
                        xt[:, h * CH:(h + 1) * CH].rearrange("(k p) t -> p k t", p=P))
                    y_sb = yp.tile([P, DKT, CH], F32, tag="y")
                    nc.vector.memzero(y_sb[:])

                    for fg in range(FBT // FG):
                        gu_g = gup.tile([P, FG, CH], F32R, tag="gu")
                        for fi in range(FG):
                            fb = fg * FG + fi
                            w1c = wp.tile([P, DKT, P], F32R, tag="w1c")
                            w3c = wp.tile([P, DKT, P], F32R, tag="w3c")
                            nc.sync.dma_start(
                                w1c[:],
                                w1t[:, fb * P:(fb + 1) * P].rearrange(
                                    "(k p) f -> p k f", p=P))
                            nc.sync.dma_start(
                                w3c[:],
                                w3t[:, fb * P:(fb + 1) * P].rearrange(
                                    "(k p) f -> p k f", p=P))
                            for (n0, nw) in nch:
                                pg = ps_g.tile([P, 512], F32, tag="pg")
                                pu = ps_g.tile([P, 512], F32, tag="pu")
                                for k in range(DKT):
                                    nc.tensor.matmul(
                                        pg[:, :nw], w1c[:, k], x_sb[:, k, n0:n0 + nw],
                                        start=(k == 0), stop=(k == DKT - 1))
                                for k in range(DKT):
                                    nc.tensor.matmul(
                                        pu[:, :nw], w3c[:, k], x_sb[:, k, n0:n0 + nw],
                                        start=(k == 0), stop=(k == DKT - 1))
                                g = gtmp.tile([P, 512], F32R, tag="g")
                                if sim_safe:
                                    # CoreSim has no Silu; sigmoid(g)*g*u instead
                                    nc.scalar.activation(
                                        g[:, :nw], pg[:, :nw],
                                        mybir.ActivationFunctionType.Sigmoid)
                                    nc.vector.tensor_mul(
                                        g[:, :nw], g[:, :nw], pg[:, :nw])
                                else:
                                    nc.scalar.activation(
                                        g[:, :nw], pg[:, :nw],
                                        mybir.ActivationFunctionType.Silu)
                                nc.vector.tensor_mul(
                                    gu_g[:, fi, n0:n0 + nw], g[:, :nw], pu[:, :nw])
                        # w2 pass for this f-group
                        w2r = wp.tile([P, FG, D], F32R, tag="w2r")
                        nc.sync.dma_start(
                            w2r[:],
                            w2t[fg * FG * P:(fg + 1) * FG * P, :].rearrange(
                                "(g p) d -> p g d", p=P))
                        for dm in range(DKT):
                            for (n0, nw) in nch:
                                py = ps_y.tile([P, 512], F32, tag="py")
                                for fi in range(FG):
                                    nc.tensor.matmul(
                                        py[:, :nw],
                                        w2r[:, fi, dm * P:(dm + 1) * P],
                                        gu_g[:, fi, n0:n0 + nw],
                                        start=(fi == 0), stop=(fi == FG - 1))
                                nc.vector.tensor_add(
                                    y_sb[:, dm, n0:n0 + nw],
                                    y_sb[:, dm, n0:n0 + nw], py[:, :nw])
                    nc.sync.dma_start(
                        y[:, h * CH:(h + 1) * CH].rearrange("(k p) t -> p k t", p=P),
                        y_sb[:])

            if reps == 1:
                body()
            else:
                with tc.For_i(0, reps, 1):
                    body()
    nc.compile()
    return nc


def _pad_to(x, n, axis=0):
    pad = [(0, 0)] * x.ndim
    pad[axis] = (0, n - x.shape[axis])
    return np.pad(x, pad)


def run_phase2(h2, tok_idx, w1, w3, w2, reps=1):
    """h2: [T, D] f32 routed input. tok_idx: list of E index arrays.
    Returns list of y_e [n_e, D] f32 (unweighted expert outputs)."""
    max_ne = max(len(ix) for ix in tok_idx)
    C = max(512, ((max_ne + 255) // 256) * 256)
    nc = build_phase2(C, reps)
    in_maps = []
    for e in range(E):
        xe = h2[tok_idx[e]]                       # [n_e, D]
        xe = _pad_to(xe, C, axis=0)               # [C, D]
        in_maps.append({
            "xt": np.ascontiguousarray(xe.T),
            "w1t": np.ascontiguousarray(w1[e].T),
            "w3t": np.ascontiguousarray(w3[e].T),
            "w2t": np.ascontiguousarray(w2[e].T),
        })
    res = bass_utils.run_bass_kernel_spmd(nc, in_maps, core_ids=list(range(NCORES)))
    outs = []
    for e in range(E):
        ye = res.results[e]["y"]                  # [D, C]
        outs.append(np.ascontiguousarray(ye.T[: len(tok_idx[e])]))
    return outs


# ---------------------------------------------------------------- phase 1
ST = S // P            # 16 seq tiles per batch
SC = S // 512          # 4 seq chunks of 512 per batch
QH = 2                 # q-heads per core
MBIG = -1.0e9          # additive causal mask value (pre 1/sqrt(HD) scaling)


@lru_cache(maxsize=None)
def build_phase1(reps=1):
    """Per-core attention slice: 2 q-heads + 1 kv-head, both batches.

    Inputs: xT [D, T] f32r (hidden transposed; ln1 folded into wqkvT),
    wqkvT [D, 512] f32r (cols: q0,q1,k,v), woT [256, D] f32r,
    cs/ss [128, T] f32r (rope tables * rmsnorm scale, positionally expanded),
    stok [128, T/128] f32 (rmsnorm scale, token-tile layout), masks
    [4, 128, 512] f32. Output: po [T, D] f32 = partial o_proj contribution.
    """
    nc = bacc.Bacc(None, target_bir_lowering=False, debug=False)
    from concourse.masks import make_identity

    with tile.TileContext(nc) as tc:
        with (
            tc.tile_pool(name="dram", bufs=1, space="DRAM") as dram,
            tc.tile_pool(name="const", bufs=1) as constp,
            tc.tile_pool(name="xs", bufs=3) as xs,
            tc.tile_pool(name="qk", bufs=1) as qkp,
            tc.tile_pool(name="rt", bufs=2) as rtp,
            tc.tile_pool(name="pb", bufs=2) as pbp,
            tc.tile_pool(name="stat", bufs=2) as statp,
            tc.tile_pool(name="oout", bufs=3) as oout,
            tc.tile_pool(name="ps_big", bufs=1, space="PSUM") as ps_big,
            tc.tile_pool(name="ps_av", bufs=2, space="PSUM") as ps_av,
            tc.tile_pool(name="ps_tr", bufs=2, space="PSUM") as ps_tr,
        ):
            xT = dram.tile([D, T], F32R, kind="ExternalInput", name="xT", uniquify=False)
            wqkvT = dram.tile([D, 4 * P], F32R, kind="ExternalInput", name="wqkvT", uniquify=False)
            woT = dram.tile([2 * P, D], F32R, kind="ExternalInput", name="woT", uniquify=False)
            cs = dram.tile([P, T], F32R, kind="ExternalInput", name="cs", uniquify=False)
            ss = dram.tile([P, T], F32R, kind="ExternalInput", name="ss", uniquify=False)
            stok = dram.tile([P, T // P], F32, kind="ExternalInput", name="stok", uniquify=False)
            masks = dram.tile([4, P, 512], F32, kind="ExternalInput", name="masks", uniquify=False)
            po = dram.tile([T, D], F32, kind="ExternalOutput", name="po", uniquify=False)

            def body():
                wq_sb = constp.tile([P, DKT, 4 * P], F32R, tag="wq")
                nc.sync.dma_start(wq_sb[:], wqkvT[:].rearrange("(k p) f -> p k f", p=P))
                wo_sb = constp.tile([P, QH, D], F32R, tag="wo")
                nc.sync.dma_start(wo_sb[:], woT[:].rearrange("(h p) d -> p h d", p=P))
                cs_sb = constp.tile([P, T], F32R, tag="cs")
                nc.sync.dma_start(cs_sb[:], cs[:])
                ss_sb = constp.tile([P, T], F32R, tag="ss")
                nc.sync.dma_start(ss_sb[:], ss[:])
                stok_sb = constp.tile([P, T // P], F32, tag="stok")
                nc.sync.dma_start(stok_sb[:], stok[:])
                mask_sb = constp.tile([P, 4, 512], F32, tag="mask")
                nc.sync.dma_start(mask_sb[:], masks[:].rearrange("m p f -> p m f"))
                ident = constp.tile([P, P], F32, tag="ident")
                make_identity(nc, ident[:])

                inv_sq = 1.0 / math.sqrt(HD)

                for b in range(B):
                    toff = b * S
                    # ---- qkv projection + rope + v transpose ----
                    q_r = [rtp.tile([P, S], F32R, tag=f"q_r{h}", bufs=1, name=f"q_r{h}") for h in range(QH)]
                    k_r = rtp.tile([P, S], F32R, tag="k_r", bufs=1)
                    v_tm = rtp.tile([P, ST, P], F32R, tag="v_tm", bufs=1)
                    for n in range(SC):
                        nsl = slice(toff + n * 512, toff + (n + 1) * 512)
                        lsl = slice(n * 512, (n + 1) * 512)
                        pq = ps_big.tile([P, 4, 512], F32, tag="big4")
                        for k in range(DKT):
                            xt = xs.tile([P, 512], F32R, tag="xt")
                            nc.sync.dma_start(xt[:], xT[k * P:(k + 1) * P, nsl])
                            for m in range(4):
                                nc.tensor.matmul(
                                    pq[:, m, :], wq_sb[:, k, m * P:(m + 1) * P], xt[:],
                                    start=(k == 0), stop=(k == DKT - 1))
                        # rope for q0, q1, k (m = 0,1,2)
                        for m in range(3):
                            dst = q_r[m][:, lsl] if m < QH else k_r[:, lsl]
                            rot = statp.tile([P, 512], F32, tag="rot")
                            nc.vector.tensor_scalar_mul(
                                rot[:64, :], pq[64:, m, :], -1.0)
                            nc.vector.tensor_copy(rot[64:, :], pq[:64, m, :])
                            tmp = statp.tile([P, 512], F32, tag="rtmp")
                            nc.vector.tensor_mul(tmp[:], rot[:], ss_sb[:, nsl])
                            nc.vector.tensor_mul(dst, pq[:, m, :], cs_sb[:, nsl])
                            nc.vector.tensor_add(dst, dst, tmp[:])
                        # v: evict, transpose to token-major, scale by stok
                        vst = statp.tile([P, 512], F32, tag="vst")
                        nc.scalar.copy(vst[:], pq[:, 3, :])
                        for j in range(4):
                            tt = n * 4 + j
                            trp = ps_tr.tile([P, P], F32, tag="tr")
                            nc.tensor.transpose(
                                trp[:], vst[:, j * P:(j + 1) * P], ident[:])
                            nc.vector.tensor_scalar_mul(
                                v_tm[:, tt, :], trp[:],
                                stok_sb[:, b * ST + tt:b * ST + tt + 1])

                    # ---- attention + o_proj, per q-group of 256 tokens ----
                    for g in range(ST // 2):
                        attn_g = [rtp.tile([P, 2 * P], F32R, tag=f"attn{h}", name=f"attn{h}")
                                  for h in range(QH)]
                        nch = g // 2 + 1          # 512-chunks covering both q-tiles
                        for h in range(QH):
                            pT = pbp.tile([P, ST, 2 * P], F32R, tag="pT", bufs=1)
                            for qi in range(2):
                                qt = 2 * g + qi
                                qsl = slice(qt * P, (qt + 1) * P)
                                sc_ps = ps_big.tile([P, 4, 512], F32, tag="big4")
                                mx = statp.tile([P, 4], F32, tag="mx")
                                for c in range(nch):
                                    nc.tensor.matmul(
                                        sc_ps[:, c, :], q_r[h][:, qsl],
                                        k_r[:, c * 512:(c + 1) * 512],
                                        start=True, stop=True)
                                    if c == nch - 1:
                                        nc.vector.tensor_add(
                                            sc_ps[:, c, :], sc_ps[:, c, :],
                                            mask_sb[:, qt % 4, :])
                                    nc.vector.tensor_reduce(
                                        mx[:, c:c + 1], sc_ps[:, c, :],
                                        axis=mybir.AxisListType.X,
                                        op=mybir.AluOpType.max)
                                m_ = statp.tile([P, 1], F32, tag="m_")
                                nc.vector.tensor_reduce(
                                    m_[:], mx[:, :nch], axis=mybir.AxisListType.X,
                                    op=mybir.AluOpType.max)
                                mneg = statp.tile([P, 1], F32, tag="mneg")
                                nc.vector.tensor_scalar_mul(mneg[:], m_[:], -inv_sq)
                                probs = pbp.tile([P, S], F32, tag="probs")
                                ls = statp.tile([P, 4], F32, tag="ls")
                                for c in range(nch):
                                    nc.scalar.activation(
                                        probs[:, c * 512:(c + 1) * 512],
                                        sc_ps[:, c, :],
                                        mybir.ActivationFunctionType.Exp,
                                        bias=mneg[:], scale=inv_sq,
                                        accum_out=ls[:, c:c + 1])
                                l_ = statp.tile([P, 1], F32, tag="l_")
                                nc.vector.tensor_reduce(
                                    l_[:], ls[:, :nch], axis=mybir.AxisListType.X,
                                    op=mybir.AluOpType.add)
                                linv = statp.tile([P, 1], F32, tag="linv")
                                nc.vector.reciprocal(linv[:], l_[:])
                                for c in range(nch):
                                    nc.vector.tensor_scalar_mul(
                                        probs[:, c * 512:(c + 1) * 512],
                                        probs[:, c * 512:(c + 1) * 512], linv[:])
                                for kt in range(4 * nch):
                                    trp = ps_tr.tile([P, P], F32, tag="tr")
                                    nc.tensor.transpose(
                                        trp[:], probs[:, kt * P:(kt + 1) * P],
                                        ident[:])
                                    nc.vector.tensor_copy(
                                        pT[:, kt, qi * P:(qi + 1) * P], trp[:])
                            av = ps_av.tile([P, 2 * P], F32, tag="av")
                            for kt in range(4 * nch):
                                nc.tensor.matmul(
                                    av[:], v_tm[:, kt, :], pT[:, kt, :],
                                    start=(kt == 0), stop=(kt == 4 * nch - 1))
                            nc.scalar.copy(attn_g[h][:], av[:])
                        # o_proj partial for this group's 2 token tiles
                        for qi in range(2):
                            tt = 2 * g + qi
                            for dn in range(4):
                                ops = ps_big.tile([P, 4, 512], F32, tag="big4")
                                for h in range(QH):
                                    nc.tensor.matmul(
                                        ops[:, 0, :],
                                        attn_g[h][:, qi * P:(qi + 1) * P],
                                        wo_sb[:, h, dn * 512:(dn + 1) * 512],
                                        start=(h == 0), stop=(h == QH - 1))
                                ot = oout.tile([P, 512], F32, tag="ot")
                                nc.vector.tensor_copy(ot[:], ops[:, 0, :])
                                nc.sync.dma_start(
                                    po[toff + tt * P:toff + (tt + 1) * P,
                                       dn * 512:(dn + 1) * 512], ot[:])

            if reps == 1:
                body()
            else:
                with tc.For_i(0, reps, 1):
                    body()
    nc.compile()
    return nc


def attention_host_prep(hidden, cos, sin, ln1_w, wqkv, wo):
    """Builds the 8 per-core input maps for phase 1."""
    x = hidden.reshape(T, D)
    x64 = x.astype(np.float64)
    s = 1.0 / np.sqrt((x64 * x64).mean(-1) + EPS)          # [T] rmsnorm scale
    s32 = s.astype(np.float32)
    xT = np.ascontiguousarray(x.T)                          # [D, T]
    wqkv_ln = (wqkv.astype(np.float64) * ln1_w.astype(np.float64)[None, :]).astype(np.float32)

    cosT = cos.T.astype(np.float64)                         # [HD, S]
    sinT = sin.T.astype(np.float64)
    pos = np.tile(np.arange(S), B)                          # position of each token
    cs = (cosT[:, pos] * s[None, :]).astype(np.float32)     # [HD, T]
    ss_ = (sinT[:, pos] * s[None, :]).astype(np.float32)
    stok = np.ascontiguousarray(s32.reshape(T // P, P).T)   # [P, T/P]

    mk = np.zeros((4, P, 512), np.float32)
    for j in range(4):
        q = np.arange(P)[:, None]
        k = np.arange(512)[None, :]
        mk[j] = np.where(k <= j * P + q, 0.0, MBIG)

    in_maps = []
    for c in range(NCORES):
        rows = np.concatenate([
            np.arange(c * QH * HD, (c * QH + QH) * HD),             # q heads
            np.arange(NH * HD + c * HD, NH * HD + (c + 1) * HD),    # k head
            np.arange((NH + NKV) * HD + c * HD,
                      (NH + NKV) * HD + (c + 1) * HD),              # v head
        ])
        wqkvT_c = np.ascontiguousarray(wqkv_ln[rows].T)             # [D, 512]
        woT_c = np.ascontiguousarray(wo[:, c * QH * HD:(c + 1) * QH * HD].T)
        in_maps.append({
            "xT": xT, "wqkvT": wqkvT_c, "woT": woT_c,
            "cs": cs, "ss": ss_, "stok": stok, "masks": mk,
        })
    return in_maps


def run_phase1(hidden, cos, sin, ln1_w, wqkv, wo, reps=1):
    """Returns attn output summed over cores: [T, D] f64."""
    nc = build_phase1(reps)
    in_maps = attention_host_prep(hidden, cos, sin, ln1_w, wqkv, wo)
    res = bass_utils.run_bass_kernel_spmd(nc, in_maps, core_ids=list(range(NCORES)))
    acc = np.zeros((T, D), np.float64)
    for c in range(NCORES):
        acc += res.results[c]["po"].astype(np.float64)
    return acc


# ---------------------------------------------------------------- routing
def route(h2_f64, gate_w):
    """Replicates reference: softmax over experts, top-2, renormalize.
    Returns tok_idx (list of E arrays) and tok_w (matching weights)."""
    logits = h2_f64 @ gate_w.astype(np.float64).T          # [T, E]
    logits -= logits.max(axis=-1, keepdims=True)
    p = np.exp(logits)
    p /= p.sum(axis=-1, keepdims=True)
    order = np.argsort(-p, axis=-1, kind="stable")[:, :TOPK]   # ties -> lower idx
    tw = np.take_along_axis(p, order, axis=-1)
    tw /= tw.sum(axis=-1, keepdims=True)
    tok_idx, tok_w = [], []
    for e in range(E):
        t_ids, k_ids = np.nonzero(order == e)
        tok_idx.append(t_ids)
        tok_w.append(tw[t_ids, k_ids])
    return tok_idx, tok_w


def moe_host(residual, gate_w, ln2_w, w1, w3, w2, reps=1):
    """Post-attention norm + router + expert dispatch. Returns out [T, D] f32."""
    r64 = residual.astype(np.float64)
    var = (r64 * r64).mean(axis=-1, keepdims=True)
    h2_64 = r64 / np.sqrt(var + EPS) * ln2_w.astype(np.float64)
    h2 = h2_64.astype(np.float32)
    tok_idx, tok_w = route(h2_64, gate_w)
    ys = run_phase2(h2, tok_idx, w1, w3, w2, reps=reps)
    out = np.zeros((T, D), np.float64)
    for e in range(E):
        np.add.at(out, tok_idx[e], tok_w[e][:, None] * ys[e].astype(np.float64))
    return out.astype(np.float32)


# ---------------------------------------------------------------- entry
def kernel(hidden_states, cos, sin, ln1_w, ln2_w, wqkv, wo, gate_w, w1, w3, w2):
    hidden_states = np.asarray(hidden_states, np.float32)
    cos = np.asarray(cos, np.float32)
    sin = np.asarray(sin, np.float32)
    ln1_w = np.asarray(ln1_w, np.float32)
    ln2_w = np.asarray(ln2_w, np.float32)
    wqkv = np.asarray(wqkv, np.float32)
    wo = np.asarray(wo, np.float32)
    gate_w = np.asarray(gate_w, np.float32)
    w1 = np.asarray(w1, np.float32)
    w3 = np.asarray(w3, np.float32)
    w2 = np.asarray(w2, np.float32)

    attn = run_phase1(hidden_states, cos, sin, ln1_w, wqkv, wo)   # [T, D] f64
    residual = (attn + hidden_states.reshape(T, D).astype(np.float64)).astype(np.float32)
    out = moe_host(residual, gate_w, ln2_w, w1, w3, w2)
    return out.reshape(B, S, D), residual.reshape(B, S, D)
